# revision 15
# baseline (speedup 1.0000x reference)
"""nn_Attention3D Trainium2 Bass kernel.

Data-parallel over batch: core b computes batch element b.
Pipeline per core (all PE inputs bf16, PSUM fp32):
  pw conv (PE) -> depthwise 3x3x3 conv (PE, block-diag Toeplitz over
  (4ch x 32n) partitions + 9 shifted passes) -> channel attention
  (Gram matmuls over DMA-transposed tiles, softmax, proj folded) -> out.
"""

import os
import sys
import numpy as np

# --- problem constants (hardcoded; kernel.py must be self-contained) ---
B, DIM, N, H, W = 8, 64, 32, 32, 32
HEADS = 8
S = N * H * W              # 32768
SH = S // 2                # 16384 (stacked half)
NCORES = 8

for _p in ("/opt/trn_rl_repo", "/root/.axon_site/_ro/trn_rl_repo"):
    if os.path.isdir(_p) and _p not in sys.path:
        sys.path.append(_p)

_RUNNER = None


def _bf16(a):
    import ml_dtypes
    return np.asarray(a, dtype=ml_dtypes.bfloat16)


def _build_nc(debug=False):
    from concourse import bacc, tile, mybir

    dt = mybir.dt
    f32, bf16 = dt.float32, dt.bfloat16

    nc = bacc.Bacc("TRN2")

    x_d = nc.declare_dram_parameter("x", [DIM, S], f32, isOutput=False)
    wqkvT_d = nc.declare_dram_parameter("wqkvT", [DIM, 3 * DIM], bf16, isOutput=False)
    toep_d = nc.declare_dram_parameter("toep", [48, 9, 128, 128], bf16, isOutput=False)
    wpT_d = nc.declare_dram_parameter("wpT", [DIM, DIM], bf16, isOutput=False)
    id128_d = nc.declare_dram_parameter("id128", [128, 128], f32, isOutput=False)
    bmask_d = nc.declare_dram_parameter("bmask", [DIM, DIM], f32, isOutput=False)
    tempc_d = nc.declare_dram_parameter("tempc", [128, 1], f32, isOutput=False)
    selk_d = nc.declare_dram_parameter("selk", [128, DIM], bf16, isOutput=False)
    ones1_d = nc.declare_dram_parameter("ones1", [1, DIM], bf16, isOutput=False)
    idA_d = nc.declare_dram_parameter("idA", [DIM, DIM], bf16, isOutput=False)
    y_d = nc.declare_dram_parameter("y", [DIM, S], f32, isOutput=True)
    if debug:
        dbg_pw = nc.declare_dram_parameter("dbg_pw", [128, SH], f32,
                                           isOutput=True)
        dbg_q = nc.declare_dram_parameter("dbg_q", [128, SH], f32,
                                          isOutput=True)
        dbg_v = nc.declare_dram_parameter("dbg_v", [128, SH], f32,
                                          isOutput=True)
        dbg_G = nc.declare_dram_parameter("dbg_G", [128, 128], f32,
                                          isOutput=True)
        dbg_rv = nc.declare_dram_parameter("dbg_rv", [128, 1], f32,
                                           isOutput=True)
        dbg_L = nc.declare_dram_parameter("dbg_L", [DIM, DIM], f32,
                                          isOutput=True)
        dbg_A = nc.declare_dram_parameter("dbg_A", [DIM, DIM], f32,
                                          isOutput=True)
        dbg_Mt = nc.declare_dram_parameter("dbg_Mt", [DIM, DIM], f32,
                                           isOutput=True)
        dbg_AT = nc.declare_dram_parameter("dbg_AT", [DIM, DIM], f32,
                                           isOutput=True)

    with tile.TileContext(nc) as tc:
        import contextlib
        ctx = contextlib.ExitStack()
        with ctx:
            cpool = ctx.enter_context(tc.tile_pool(name="const", bufs=1))
            bigp = ctx.enter_context(tc.tile_pool(name="big", bufs=4))
            qkTp = ctx.enter_context(tc.tile_pool(name="qkT", bufs=1))
            dwinp = ctx.enter_context(tc.tile_pool(name="dwin", bufs=1))
            toepp = ctx.enter_context(tc.tile_pool(name="toep", bufs=2))
            stp = ctx.enter_context(tc.tile_pool(name="stage", bufs=2))
            ystp = ctx.enter_context(tc.tile_pool(name="ystage", bufs=2))
            psA = ctx.enter_context(
                tc.tile_pool(name="psA", bufs=2, space="PSUM"))
            psB = ctx.enter_context(
                tc.tile_pool(name="psB", bufs=2, space="PSUM"))

            # ---------------- constants ----------------
            wT = cpool.tile([128, 3 * DIM], bf16)
            nc.sync.dma_start(wT[0:64, :], wqkvT_d[:, :])
            nc.sync.dma_start(wT[64:128, :], wqkvT_d[:, :])
            wpT_sb = cpool.tile([DIM, DIM], bf16)
            nc.sync.dma_start(wpT_sb[:, :], wpT_d[:, :])
            id128_sb = cpool.tile([128, 128], f32)
            nc.sync.dma_start(id128_sb[:, :], id128_d[:, :])
            bmask_sb = cpool.tile([DIM, DIM], f32)
            nc.sync.dma_start(bmask_sb[:, :], bmask_d[:, :])
            tempc_sb = cpool.tile([128, 1], f32)
            nc.sync.dma_start(tempc_sb[:, :], tempc_d[:, :])
            selk_sb = cpool.tile([128, DIM], bf16)
            nc.sync.dma_start(selk_sb[:, :], selk_d[:, :])
            ones1_sb = cpool.tile([1, DIM], bf16)
            nc.sync.dma_start(ones1_sb[:, :], ones1_d[:, :])
            idA_sb = cpool.tile([DIM, DIM], bf16)
            nc.sync.dma_start(idA_sb[:, :], idA_d[:, :])

            # persistent padded depthwise input tiles (zeroed once; pads
            # stay zero, interior rewritten per group)
            dwin = [dwinp.tile([128, 34, 34], bf16, tag=f"dwin{i}",
                                       name=f"dwin{i}") for i in range(2)]
            for t in dwin:
                nc.vector.memset(t[:, :, :], 0.0)

            # ---------------- load x + cast to bf16 ----------------
            # stacked layout: partition p<64 = channel p, s in [0,16384);
            # p>=64 = channel p-64, s in [16384,32768)
            xb = bigp.tile([128, SH], bf16, tag="big")
            with tc.tile_pool(name="x32", bufs=2) as x32p:
                CH = 1024
                for i in range(SH // CH):
                    xf = x32p.tile([128, CH], f32)
                    nc.sync.dma_start(xf[0:64, :], x_d[:, i * CH:(i + 1) * CH])
                    nc.sync.dma_start(xf[64:128, :],
                                      x_d[:, SH + i * CH: SH + (i + 1) * CH])
                    nc.vector.tensor_copy(xb[:, i * CH:(i + 1) * CH], xf[:, :])

            # ---------------- pointwise conv (3 m-tiles of 64) -------------
            pw = []
            evac_rot = 0

            def evac(dst_ap, src_ap):
                nonlocal evac_rot
                if evac_rot % 2 == 0:
                    nc.vector.tensor_copy(dst_ap, src_ap)
                else:
                    nc.scalar.copy(dst_ap, src_ap)
                evac_rot += 1

            for m in range(3):
                pwm = bigp.tile([128, SH], bf16, tag="big")
                pw.append(pwm)
                for j in range(SH // 512):
                    ps = psA.tile([128, 512], f32, tag="ps")
                    nc.tensor.matmul(ps[0:64, :],
                                     wT[0:64, m * 64:(m + 1) * 64],
                                     xb[0:64, j * 512:(j + 1) * 512])
                    nc.tensor.matmul(ps[64:128, :],
                                     wT[64:128, m * 64:(m + 1) * 64],
                                     xb[64:128, j * 512:(j + 1) * 512])
                    evac(pwm[:, j * 512:(j + 1) * 512], ps[:, :])

            # ---------------- depthwise conv + shuffles ----------------
            chan = []   # channel-major dw outputs (stacked like xb)
            for m in range(3):
                cm = bigp.tile([128, SH], bf16, tag="big")
                chan.append(cm)
                pwm = pw[m]
                for gl in range(16):
                    g = m * 16 + gl
                    tg = toepp.tile([128, 9, 128], bf16)
                    nc.sync.dma_start(
                        tg[:, :, :], toep_d[g].rearrange("p i o -> i p o"))
                    dwt = dwin[gl % 2]
                    dwt_r = dwt[:, :, :].rearrange(
                        "(c n) y x -> c n y x", c=4)
                    # shuffle in: per-channel DMAs (AP balancer: <=3 dims)
                    for nh in range(2):
                        for cl in range(4):
                            a = nh * 64 + 4 * gl + cl
                            srcp = pwm[a:a + 1, :]
                            nc.sync.dma_start(
                                dwt[cl * 32 + nh * 16: cl * 32 + nh * 16 + 16,
                                    1:33, 1:33],
                                srcp.rearrange("c (n y x) -> c n y x",
                                               n=16, y=32))
                    dwps = [psB.tile([128, 512], f32, name=f"dwps{_i}")
                            for _i in range(2)]
                    for p in range(9):
                        dy, dx = p // 3, p % 3
                        for hh in range(2):
                            rhs = dwt[:, 16 * hh + dy: 16 * hh + dy + 16,
                                      dx: dx + 32]
                            nc.tensor.matmul(dwps[hh][:, :], tg[:, p, :], rhs,
                                             start=(p == 0), stop=(p == 8))
                    for hh in range(2):
                        st = stp.tile([128, 512], bf16)
                        evac(st[:, :], dwps[hh][:, :])
                        cm_r = cm.rearrange("p (n y x) -> p n y x", n=16, y=32)
                        st_r = st[:, :].rearrange("p (y x) -> p y x", y=16)
                        for nh in range(2):
                            for cl in range(4):
                                a = nh * 64 + 4 * gl + cl
                                nc.sync.dma_start(
                                    cm_r[a:a + 1, :,
                                         hh * 16:(hh + 1) * 16, :],
                                    st_r[cl * 32 + nh * 16:
                                         cl * 32 + nh * 16 + 16, :, :])

            qchan, kchan, vchan = chan
            if debug:
                with tc.tile_pool(name="dbgp", bufs=2) as dbgp:
                    for nm, tl in (("dbg_pw", pw[0]), ("dbg_q", qchan),
                                   ("dbg_v", vchan)):
                        dd = {"dbg_pw": dbg_pw, "dbg_q": dbg_q,
                              "dbg_v": dbg_v}[nm]
                        for j in range(SH // 2048):
                            dt32 = dbgp.tile([128, 2048], f32, tag="dbg",
                                             name=f"{nm}{j}")
                            nc.vector.tensor_copy(
                                dt32[:, :], tl[:, j * 2048:(j + 1) * 2048])
                            nc.sync.dma_start(
                                dd[:, j * 2048:(j + 1) * 2048], dt32[:, :])

            # ---------------- transposes: (64, s) -> (s-tiles, 64) ----------
            # ---------------- Gram matmul: [q|k]^T [q|k] ----------------
            gps = psA.tile([128, 128], f32, tag="gps", bufs=1)
            for half in range(2):
                qkT = qkTp.tile([128, 128, 128], bf16, tag="qkT",
                                name=f"qkT{half}")
                nc.sync.dma_start(
                    qkT[:, :, 0:64],
                    qchan[half * 64:(half + 1) * 64, :], transpose=True)
                nc.sync.dma_start(
                    qkT[:, :, 64:128],
                    kchan[half * 64:(half + 1) * 64, :], transpose=True)
                for t in range(128):
                    nc.tensor.matmul(gps[:, :], qkT[:, t, :], qkT[:, t, :],
                                     start=(half == 0 and t == 0),
                                     stop=(half == 1 and t == 127))
            G = cpool.tile([128, 128], f32, tag="G")
            nc.vector.tensor_copy(G[:, :], gps[:, :])
            if debug:
                nc.sync.dma_start(dbg_G[:, :], G[:, :])

            # diag -> norms -> rsqrt (temp folded for q rows)
            dtmp = cpool.tile([128, 128], f32, tag="dtmp")
            nc.vector.tensor_tensor(dtmp[:, :], G[:, :], id128_sb[:, :],
                                    mybir.AluOpType.mult)
            dvec = cpool.tile([128, 1], f32, tag="dvec")
            nc.vector.tensor_reduce(dvec[:, :], dtmp[:, :],
                                    mybir.AxisListType.X,
                                    mybir.AluOpType.add)
            sq = cpool.tile([128, 1], f32, tag="sq")
            nc.scalar.sqrt(sq[:, :], dvec[:, :])
            rv = cpool.tile([128, 1], f32, tag="rv")
            nc.vector.reciprocal(rv[:, :], sq[:, :])
            nc.vector.tensor_tensor(rv[:, :], rv[:, :], tempc_sb[:, :],
                                    mybir.AluOpType.mult)
            if debug:
                nc.sync.dma_start(dbg_rv[:, :], rv[:, :])
            rv_bf = cpool.tile([128, 1], bf16, tag="rvbf")
            nc.vector.tensor_copy(rv_bf[:, :], rv[:, :])

            # row-scale L = G[q,k] * rq
            L = cpool.tile([DIM, DIM], f32, tag="L")
            nc.vector.tensor_scalar(L[:, :], G[0:64, 64:128], rv[0:64, :],
                                    None, mybir.AluOpType.mult)

            # col-scale by rk: extract rk as a row then replicate
            rk_ps = psB.tile([1, DIM], f32, tag="attnps", bufs=1)
            nc.tensor.matmul(rk_ps[:, :], rv_bf[:, :], selk_sb[:, :])
            rk_bf = cpool.tile([1, DIM], bf16, tag="rkbf")
            nc.vector.tensor_copy(rk_bf[:, :], rk_ps[:, :])
            rep_ps = psB.tile([DIM, DIM], f32, tag="attnps", bufs=1)
            nc.tensor.matmul(rep_ps[:, :], ones1_sb[:, :], rk_bf[:, :])
            rep = cpool.tile([DIM, DIM], f32, tag="rep")
            nc.vector.tensor_copy(rep[:, :], rep_ps[:, :])
            nc.vector.tensor_tensor(L[:, :], L[:, :], rep[:, :],
                                    mybir.AluOpType.mult)

            if debug:
                nc.sync.dma_start(dbg_L[:, :], L[:, :])
            # softmax over free axis within heads
            expL = cpool.tile([DIM, DIM], f32, tag="expL")
            nc.scalar.activation(expL[:, :], L[:, :],
                                 mybir.ActivationFunctionType.Exp)
            nc.vector.tensor_tensor(expL[:, :], expL[:, :], bmask_sb[:, :],
                                    mybir.AluOpType.mult)
            ssum = cpool.tile([DIM, 1], f32, tag="ssum")
            nc.vector.tensor_reduce(ssum[:, :], expL[:, :],
                                    mybir.AxisListType.X,
                                    mybir.AluOpType.add)
            rs = cpool.tile([DIM, 1], f32, tag="rs")
            nc.vector.reciprocal(rs[:, :], ssum[:, :])
            nc.vector.tensor_scalar(expL[:, :], expL[:, :], rs[:, :], None,
                                    mybir.AluOpType.mult)
            if debug:
                nc.sync.dma_start(dbg_A[:, :], expL[:, :])
            A_bf = cpool.tile([DIM, DIM], bf16, tag="Abf")
            nc.vector.tensor_copy(A_bf[:, :], expL[:, :])

            # Mt = (wp @ A)^T = A^T @ wp^T  (lhsT = A directly)
            m_ps = psB.tile([DIM, DIM], f32, tag="attnps", bufs=1)
            nc.tensor.matmul(m_ps[:, :], A_bf[:, :], wpT_sb[:, :])
            Mt = cpool.tile([128, DIM], bf16, tag="Mt")
            nc.vector.tensor_copy(Mt[0:64, :], m_ps[:, :])
            if debug:
                mt32 = cpool.tile([DIM, DIM], f32, tag="mt32")
                nc.vector.tensor_copy(mt32[:, :], m_ps[:, :])
                nc.sync.dma_start(dbg_Mt[:, :], mt32[:, :])
            nc.sync.dma_start(Mt[64:128, :], Mt[0:64, :])

            # ---------------- out = Mt^T @ v, stream + store ----------------
            for j in range(SH // 512):
                ps = psA.tile([128, 512], f32, tag="ps")
                nc.tensor.matmul(ps[0:64, :], Mt[0:64, :],
                                 vchan[0:64, j * 512:(j + 1) * 512])
                nc.tensor.matmul(ps[64:128, :], Mt[64:128, :],
                                 vchan[64:128, j * 512:(j + 1) * 512])
                yst = ystp.tile([128, 512], f32, tag="yst")
                evac(yst[:, :], ps[:, :])
                nc.sync.dma_start(y_d[:, j * 512:(j + 1) * 512], yst[0:64, :])
                nc.sync.dma_start(y_d[:, SH + j * 512: SH + (j + 1) * 512],
                                  yst[64:128, :])

    nc.compile()
    return nc


def _host_prep(x, w_qkv, w_dw, w_proj, temperature):
    import ml_dtypes
    bf = ml_dtypes.bfloat16
    wq = w_qkv[:, :, 0, 0, 0].astype(np.float32)          # (192, 64)
    wd = w_dw[:, 0].astype(np.float32)                    # (192, 3, 3, 3)
    wp = w_proj[:, :, 0, 0, 0].astype(np.float32)         # (64, 64)
    temp = np.asarray(temperature, np.float32).reshape(-1)[:B]

    wqkvT = wq.T.astype(bf)                               # (64, 192)
    wpT = wp.T.astype(bf)                                 # (64, 64)

    # Toeplitz: toep[g, p=(dy*3+dx), cl*32+ni, cl*32+no] =
    #   wd[4g+cl, dz=ni-no+1, dy, dx]
    eye3 = np.stack([np.eye(32, k=1 - dz, dtype=np.float32)
                     for dz in range(3)])                 # (3, 32, 32)
    wd2 = wd.reshape(48, 4, 3, 9)                         # g, cl, dz, p
    blk = np.einsum('gczp,zio->gpcio', wd2, eye3)         # (48, 9, 4, 32, 32)
    toep = np.zeros((48, 9, 128, 128), np.float32)
    for cl in range(4):
        toep[:, :, cl * 32:(cl + 1) * 32, cl * 32:(cl + 1) * 32] = \
            blk[:, :, cl]
    toep = toep.astype(bf)

    id128 = np.eye(128, dtype=np.float32)
    bmask = np.kron(np.eye(HEADS, dtype=np.float32),
                    np.ones((8, 8), np.float32))          # (64, 64)
    selk = np.zeros((128, DIM), np.float32)
    selk[64:128, :] = np.eye(64)
    selk = selk.astype(bf)
    ones1 = np.ones((1, DIM), np.float32).astype(bf)
    idA = np.eye(DIM, dtype=np.float32).astype(bf)

    shared = dict(wqkvT=wqkvT, toep=toep, wpT=wpT, id128=id128,
                  bmask=bmask, selk=selk, ones1=ones1, idA=idA)
    in_maps = []
    for b in range(B):
        tempc = np.full((128, 1), 1.0, np.float32)
        tempc[0:64, 0] = temp[b]
        m = dict(shared)
        m["x"] = np.ascontiguousarray(
            x[b].reshape(DIM, S).astype(np.float32))
        m["tempc"] = tempc
        in_maps.append(m)
    return in_maps


class _Runner:
    """Builds the Bass program + jitted SPMD executable once; reuses it."""

    def __init__(self):
        import jax
        self.jax = jax
        self.nc = _build_nc()
        self._build_jit()

    def _build_jit(self):
        import jax
        import jax.numpy as jnp
        from jax.sharding import Mesh, PartitionSpec
        from jax.experimental.shard_map import shard_map
        from concourse import bass2jax, mybir
        from concourse.bass2jax import (_bass_exec_p, install_neuronx_cc_hook,
                                        partition_id_tensor)

        install_neuronx_cc_hook()
        nc = self.nc

        partition_name = (nc.partition_id_tensor.name
                          if nc.partition_id_tensor else None)
        in_names, out_names, out_avals, zero_shapes = [], [], [], []
        for alloc in nc.m.functions[0].allocations:
            if not isinstance(alloc, mybir.MemoryLocationSet):
                continue
            name = alloc.memorylocations[0].name
            if alloc.kind == "ExternalInput":
                if name != partition_name:
                    in_names.append(name)
            elif alloc.kind == "ExternalOutput":
                shape = tuple(alloc.tensor_shape)
                dtype = mybir.dt.np(alloc.dtype)
                out_avals.append(jax.core.ShapedArray(shape, dtype))
                out_names.append(name)
                zero_shapes.append((shape, dtype))
        self.in_names = list(in_names)
        self.out_names = list(out_names)
        self.zero_shapes = zero_shapes
        n_params = len(in_names)
        n_outs = len(out_names)
        all_in = list(in_names) + list(out_names)
        if partition_name is not None:
            all_in.append(partition_name)

        def _body(*args):
            operands = list(args)
            if partition_name is not None:
                operands.append(partition_id_tensor())
            outs = _bass_exec_p.bind(
                *operands,
                out_avals=tuple(out_avals),
                in_names=tuple(all_in),
                out_names=tuple(out_names),
                lowering_input_output_aliases=(),
                sim_require_finite=False,
                sim_require_nnan=False,
                nc=nc,
            )
            return tuple(outs)

        devices = jax.devices()[:NCORES]
        mesh = Mesh(np.asarray(devices), ("core",))
        in_specs = (PartitionSpec("core"),) * (n_params + n_outs)
        out_specs = (PartitionSpec("core"),) * n_outs
        donate = tuple(range(n_params, n_params + n_outs))
        self.jitted = jax.jit(
            shard_map(_body, mesh=mesh, in_specs=in_specs,
                      out_specs=out_specs, check_rep=False),
            donate_argnums=donate, keep_unused=True)

    def run(self, in_maps):
        np_ = np
        concat_in = [
            np_.concatenate([np_.asarray(in_maps[c][name])
                             for c in range(NCORES)], axis=0)
            for name in self.in_names
        ]
        concat_zeros = [
            np_.zeros((NCORES * s[0], *s[1:]), d)
            for (s, d) in self.zero_shapes
        ]
        outs = self.jitted(*concat_in, *concat_zeros)
        res = np_.asarray(outs[0])
        per = res.reshape(NCORES, *self.zero_shapes[0][0])
        return per


def kernel(x, w_qkv, w_dw, w_proj, temperature):
    global _RUNNER
    x = np.asarray(x, np.float32)
    in_maps = _host_prep(x, np.asarray(w_qkv), np.asarray(w_dw),
                         np.asarray(w_proj), np.asarray(temperature))
    if _RUNNER is None:
        _RUNNER = _Runner()
    per = _RUNNER.run(in_maps)            # (8, 64, 32768)
    return np.ascontiguousarray(per.reshape(B, DIM, N, H, W))
